# revision 41
# baseline (speedup 1.0000x reference)
"""Trainium2 Bass kernel for the Bolt 64-QAM demapper MLP forward pass.

Problem: llr = (relu(relu(z @ W1 + b1) @ W2 + b2) @ W3 + b3).reshape(B, S*6)
  z [4096, 512, 3] f32, W1 [3,128], W2 [128,128], W3 [128,6].

Strategy: pure data parallel over 8 NeuronCores (batch split). The on-chip
critical path is shared between the PE (12 matmuls/tile at the throttled
1.2 GHz clock) and the PSUM->SBUF evacuation of the activations (only ACT
and DVE can read PSUM, both at 1 elem/lane/cycle), so the design:

  - host pre-transposes z to a feature-major, PE-band-interleaved bf16
    layout so one band-sliced DMA per 8192-row quad feeds L1 directly
    (no on-chip expand/transpose work at all),
  - L1 runs as 4 concurrent row-packed K=3 matmuls (PE row bands), L2 as
    4 serial full-array K=128 matmuls, L3 as 4 concurrent col-packed
    M=32 matmuls,
  - ACT evacuates h1 (fused relu+bias+bf16 cast) and the output (fused
    +b3 via Identity); DVE evacuates all four h2 chunks (fused relu+b2),
  - the output is stored feature-major bf16 (band-sliced DMA) and the
    host reassembles/casts to the row-major f32 result,
  - stages are software-pipelined (step t runs L1(t) | L3(t-2)+out |
    L2(t-1)+evac) so each engine always has runnable work; z loads use
    the gpsimd SWDGE path so output stores never head-of-line block
    them on the Sync engine's HWDGE FIFO.

Row mapping: row = 8192*q + 2048*j + 512*a + n  (q quad, j tile-in-quad,
a PE band, n<512). PSUM: h1 [128,2048] + 3x h2 [128,512] + out [128,512]
= 8 banks exactly.
"""
import os
import numpy as np
from contextlib import ExitStack

import concourse.bacc as bacc
import concourse.mybir as mybir
import concourse.tile as tile
from concourse import bass_utils
from bass_rust import add_dep_helper

F32 = mybir.dt.float32
BF16 = mybir.dt.bfloat16
AF = mybir.ActivationFunctionType
ALU = mybir.AluOpType

N_CORES = 8
B, S, H, NB = 4096, 512, 128, 6
ROWS_TOTAL = B * S                    # 2097152
ROWS_CORE = ROWS_TOTAL // N_CORES     # 262144
TROWS = 2048                          # rows per tile
NT = ROWS_CORE // TROWS               # 128 tiles
NQ = NT // 4                          # 32 quads

LAST_RESULTS = None  # stashed BassKernelResults for test harness inspection


def _build_nc():
    nc = bacc.Bacc("TRN2", target_bir_lowering=False, debug=False, num_devices=N_CORES)
    # z, feature-major band-interleaved: [q, band a, feat u, tile j, n]
    z_d = nc.dram_tensor("zt", [NQ, 4, 3, 4, 512], BF16, kind="ExternalInput")
    w1rep_d = nc.dram_tensor("w1rep", [128, H], BF16, kind="ExternalInput")
    b1_d = nc.dram_tensor("b1", [H, 1], F32, kind="ExternalInput")
    w2_d = nc.dram_tensor("w2", [H, H], BF16, kind="ExternalInput")
    b2_d = nc.dram_tensor("b2", [H, 1], F32, kind="ExternalInput")
    w3_d = nc.dram_tensor("w3", [H, 32], BF16, kind="ExternalInput")
    b3rep_d = nc.dram_tensor("b3rep", [128, 1], F32, kind="ExternalInput")
    # out, feature-major band-sliced: [q, band c, feat u, tile j, n]
    out_d = nc.dram_tensor("out", [NQ, 4, NB, 4, 512], BF16, kind="ExternalOutput")

    with tile.TileContext(nc) as tc, ExitStack() as ctx:
        const = ctx.enter_context(tc.tile_pool(name="const", bufs=1))
        zp = ctx.enter_context(tc.tile_pool(name="zp", bufs=6))
        h1p = ctx.enter_context(tc.tile_pool(name="h1p", bufs=3))
        h2p = ctx.enter_context(tc.tile_pool(name="h2p", bufs=3))
        op = ctx.enter_context(tc.tile_pool(name="op", bufs=3))
        ps_h1 = ctx.enter_context(tc.tile_pool(name="ps_h1", bufs=1, space="PSUM"))
        ps_h2 = ctx.enter_context(tc.tile_pool(name="ps_h2", bufs=3, space="PSUM"))
        ps_o = ctx.enter_context(tc.tile_pool(name="ps_o", bufs=1, space="PSUM"))

        w1rep = const.tile([128, H], BF16)
        nc.sync.dma_start(w1rep[:], w1rep_d.ap())
        w2sb = const.tile([H, H], BF16)
        nc.sync.dma_start(w2sb[:], w2_d.ap())
        w3sb = const.tile([H, 32], BF16)
        nc.sync.dma_start(w3sb[:], w3_d.ap())
        b1sb = const.tile([H, 1], F32)
        nc.sync.dma_start(b1sb[:], b1_d.ap())
        b2sb = const.tile([H, 1], F32)
        nc.sync.dma_start(b2sb[:], b2_d.ap())
        b3sb = const.tile([128, 1], F32)
        nc.sync.dma_start(b3sb[:], b3rep_d.ap())

        # PE instruction-stream adjacency chains (keeps packed groups dense)
        last_mm = [None]

        def group():
            last_mm[0] = None

        def mm(*args, **kw):
            inst = nc.tensor.matmul(*args, **kw)
            if last_mm[0] is not None:
                add_dep_helper(inst.ins, last_mm[0].ins, False, "pe group order")
            last_mm[0] = inst
            return inst

        zsbs, h1sbs, h2ps, h2sbs, outps, outsbs = {}, {}, {}, {}, {}, {}

        def stage_load(q):
            # SWDGE (gpsimd) path: keeps the Sync engine's HWDGE FIFO free
            # for output stores (avoids head-of-line blocking).
            zsb = zp.tile([128, 2048], BF16, tag="z", name="zsb")
            for a in range(4):
                nc.gpsimd.dma_start(zsb[32 * a : 32 * a + 3, :], z_d.ap()[q, a])
            zsbs[q] = zsb

        def stage_l1(t):
            q, j = divmod(t, 4)
            zsb = zsbs[q]
            h1_ps = ps_h1.tile([128, 2048], F32, name="h1ps")
            group()
            for a in (1, 2, 3, 0):
                mm(
                    h1_ps[:, 512 * a : 512 * (a + 1)],
                    w1rep[32 * a : 32 * a + 3, :],
                    zsb[32 * a : 32 * a + 3, 512 * j : 512 * (j + 1)],
                    tile_position=(32 * a, 0),
                )
            h1_sb = h1p.tile([128, 2048], BF16, tag="h1", name="h1sb")
            nc.scalar.activation(h1_sb[:], h1_ps[:], AF.Relu, bias=b1sb[:])
            h1sbs[t] = h1_sb

        def stage_l2(t):
            h1_sb = h1sbs.pop(t)
            pss = []
            group()
            for k in range(4):
                h2_ps = ps_h2.tile([128, 512], F32, name="h2ps")
                mm(h2_ps[:], w2sb[:], h1_sb[:, 512 * k : 512 * (k + 1)])
                pss.append(h2_ps)
            h2ps[t] = pss

        def evac_h2(t):
            pss = h2ps.pop(t)
            h2_sb = h2p.tile([128, 2048], BF16, tag="h2", name="h2sb")
            # all four chunks on DVE (ACT handles h1 + out)
            for k in range(4):
                nc.vector.tensor_scalar(
                    h2_sb[:, 512 * k : 512 * (k + 1)],
                    pss[k][:],
                    b2sb[:],
                    0.0,
                    op0=ALU.add,
                    op1=ALU.max,
                )
            h2sbs[t] = h2_sb

        def stage_l3(t):
            h2_sb = h2sbs.pop(t)
            out_ps = ps_o.tile([128, 512], F32, tag="o", name="ops")
            group()
            for c in range(4):
                mm(
                    out_ps[32 * c : 32 * c + 32, :],
                    w3sb[:],
                    h2_sb[:, 512 * c : 512 * (c + 1)],
                    tile_position=(0, 32 * c),
                )
            outps[t] = out_ps

        def evac_out(t):
            q, j = divmod(t, 4)
            out_ps = outps.pop(t)
            if j == 0:
                outsbs[q] = op.tile([128, 2048], BF16, tag="o", name="outsb")
            outsb = outsbs[q]
            # ACT: identity activation with per-partition b3 bias, bf16 out
            nc.scalar.activation(
                outsb[:, 512 * j : 512 * (j + 1)], out_ps[:], AF.Identity, bias=b3sb[:]
            )
            if j == 1 or j == 3:
                # store in two waves per quad: halves the store latency
                # exposed at the pipeline tail and smooths the Sync queue
                h = j // 2
                for c in range(4):
                    nc.sync.dma_start(
                        out_d.ap()[q, c][:, 2 * h : 2 * h + 2, :],
                        outsb[32 * c : 32 * c + NB, 1024 * h : 1024 * (h + 1)],
                    )
                if j == 3:
                    outsbs.pop(q)

        # software pipeline: step t runs L1(t) | L2(t-1)+evac | L3(t-2)+out
        for q in range(4):
            stage_load(q)
        for t in range(NT + 2):
            if t < NT:
                stage_l1(t)
            if t >= 2:
                stage_l3(t - 2)
                evac_out(t - 2)
            if 1 <= t <= NT:
                stage_l2(t - 1)
                evac_h2(t - 1)
            if t % 4 == 0 and (t // 4 + 4) < NQ:
                stage_load(t // 4 + 4)

    nc.compile()
    return nc


def kernel(z, W1, b1, W2, b2, W3, b3):
    global LAST_RESULTS
    z = np.asarray(z, dtype=np.float32)
    W1 = np.asarray(W1, dtype=np.float32)
    b1 = np.asarray(b1, dtype=np.float32)
    W2 = np.asarray(W2, dtype=np.float32)
    b2 = np.asarray(b2, dtype=np.float32)
    W3 = np.asarray(W3, dtype=np.float32)
    b3 = np.asarray(b3, dtype=np.float32)
    nbf = mybir.dt.np(BF16)

    # host-side weight prep (tiny)
    w1rep = np.zeros((128, H), nbf)
    for a in range(4):
        w1rep[32 * a : 32 * a + 3] = W1.astype(nbf)
    w3pad = np.zeros((H, 32), nbf)
    w3pad[:, :NB] = W3.astype(nbf)
    b3rep = np.zeros((128, 1), np.float32)
    for c in range(4):
        b3rep[32 * c : 32 * c + NB, 0] = b3

    # host-side z layout: [q, band a, feat u, tile j, n] per core
    z_rows = np.ascontiguousarray(z).reshape(ROWS_TOTAL, 3)
    zb = z_rows.astype(nbf).reshape(N_CORES, NQ, 4, 4, 512, 3)  # [c,q,j,a,n,u]
    zt = np.ascontiguousarray(zb.transpose(0, 1, 3, 5, 2, 4))   # [c,q,a,u,j,n]

    common = {
        "w1rep": w1rep,
        "b1": np.ascontiguousarray(b1.reshape(H, 1)),
        "w2": np.ascontiguousarray(W2.astype(nbf)),
        "b2": np.ascontiguousarray(b2.reshape(H, 1)),
        "w3": w3pad,
        "b3rep": b3rep,
    }
    in_maps = [dict(common, zt=np.ascontiguousarray(zt[c])) for c in range(N_CORES)]

    nc = _build_nc()
    res = bass_utils.run_bass_kernel_spmd(
        nc,
        in_maps,
        core_ids=list(range(N_CORES)),
        trace=bool(os.environ.get("KBENCH_TRACE")),
    )
    LAST_RESULTS = res
    # out: [q, c, u, j, n] bf16 -> row-major [rows, 6] f32
    outs = []
    for i in range(N_CORES):
        o = res.results[i]["out"]  # [NQ, 4, NB, 4, 512] bf16
        o = np.asarray(o).transpose(0, 3, 1, 4, 2)  # [q, j, c, n, u]
        outs.append(o.reshape(ROWS_CORE, NB))
    full = np.concatenate(outs, axis=0).astype(np.float32)
    return full.reshape(B, S * NB)
